# revision 8
# baseline (speedup 1.0000x reference)
"""Multi-head causal attention (dense transformer block) on 8 TRN2 NeuronCores.

Problem: B=2, S=2048, D_MODEL=768, H=12 heads, D_HEAD=64, fp32 I/O.

Sharding: 24 (batch, head) units over 8 cores -> 3 heads x 1 batch per core.
Cores 0-3 handle batch 0 (heads 0-2, 3-5, 6-8, 9-11), cores 4-7 batch 1.
Each core computes its heads' contribution to out[b] = sum_h z_h @ W_O[h];
the host sums the 4 partials per batch and adds b_O.

Per-core dataflow (all matmuls bf16 -> fp32 PSUM):
  - x[b]^T staged in SBUF as 6 chunks [128, 2048].
  - QT/KT in "head-transposed" layout [64, S] packed 2-per-tile:
      QT01 = [Q_h0*s | Q_h1*s]^T x  (scale 1/sqrt(64) folded into W_Q/b_Q)
      KT01 = [K_h0 | K_h1]^T x, QKT2 = [Q_h2*s | K_h2]^T x
    QK2x = [K_h2 (rows 0-63) | Q_h2*s (rows 64-127)] partition-shift copies so
    h2 score matmuls for consecutive k-tiles run on disjoint row halves.
  - scores TRANSPOSED: sT[k, q] = KT^T-slice x QT (K=64 contraction). Each
    PSUM score tile [128, 1024] holds BOTH halves of a row-group pair
    ([h0_j | h1_j], or [h2_j | h2_j+1]) so the two K=64 matmuls are released
    together and execute CONCURRENTLY in the PE array (row-group tiling,
    ~2x measured).
  - exp via ScalarE (one instr per [128, 1024] tile), causal mask via
    memset + triangular-mask multiply on diagonal 128x128 tiles (DVE).
  - zT[h] = sum_j [V_h[j] | 1]^T x PT[j]  (unnormalized; the ones column
    makes output row 64 the softmax denominator), normalized as
    zT * broadcast(1/sums) when copying PSUM->SBUF.
  - output projection interleaved per q-block (overlaps the next block's
    ACT-bound attention): out[q, :] = zT01^T-slice @ WO01 + zT2^T @ WO2,
    with the K=64 zT2 matmuls of adjacent q-tiles row-group paired via a
    high-half copy of zT2.
"""

import numpy as np
import ml_dtypes
from contextlib import ExitStack

import concourse.bass as bass
import concourse.mybir as mybir
import concourse.tile as tile
from concourse import bacc
from concourse.bass_utils import run_bass_kernel_spmd

BF16 = mybir.dt.bfloat16
F32 = mybir.dt.float32
AF = mybir.ActivationFunctionType
NPBF16 = ml_dtypes.bfloat16

B, S, D, H, DH = 2, 2048, 768, 12, 64
N_CORES = 8
DCH = D // 128          # 6 d_model chunks
NKT = S // 128          # 16 k tiles
QB = 512                # q block width
NQB = S // QB           # 4 q blocks

TRACE_ENABLED = False
LAST_EXEC_NS = None
LAST_RESULT = None
_BUILT = None


def build_nc():
    nc = bacc.Bacc("TRN2", target_bir_lowering=False, debug=False)

    xT_d = nc.dram_tensor("xT", [D, S], BF16, kind="ExternalInput")
    wq01_d = nc.dram_tensor("wq01", [D, 128], BF16, kind="ExternalInput")
    wk01_d = nc.dram_tensor("wk01", [D, 128], BF16, kind="ExternalInput")
    wqk2_d = nc.dram_tensor("wqk2", [D, 128], BF16, kind="ExternalInput")
    wv_d = nc.dram_tensor("wv", [D, 195], BF16, kind="ExternalInput")
    wo01_d = nc.dram_tensor("wo01", [128, D], BF16, kind="ExternalInput")
    wo2_d = nc.dram_tensor("wo2", [128, D], BF16, kind="ExternalInput")
    bq01_d = nc.dram_tensor("bq01", [128, 1], F32, kind="ExternalInput")
    bk01_d = nc.dram_tensor("bk01", [128, 1], F32, kind="ExternalInput")
    bqk2_d = nc.dram_tensor("bqk2", [128, 1], F32, kind="ExternalInput")
    bv_d = nc.dram_tensor("bv", [128, 195], F32, kind="ExternalInput")
    out_d = nc.dram_tensor("out_p", [S, D], F32, kind="ExternalOutput")

    tri_np = np.triu(np.ones((128, 128), np.float32)).astype(NPBF16)
    tri_d = nc.inline_tensor(tri_np, "tri")

    with tile.TileContext(nc) as tc, ExitStack() as ctx:
        persist = ctx.enter_context(tc.tile_pool(name="persist", bufs=1))

        # ---- HAM warm-up: dummy matmuls spanning the input-DMA wait, so
        # the PE clock is ramping while xT lands ----
        with tc.tile_pool(name="warm_ps", bufs=1, space="PSUM") as warm_pool:
            wz = persist.tile([128, 128], BF16, tag="wz")
            nc.vector.memset(wz[:], 0.0)
            wps = warm_pool.tile([128, 128], F32, tag="warm")
            for _ in range(36):
                nc.tensor.matmul(wps[:], wz[:], wz[:], start=True, stop=True)

        # ---- stage inputs in SBUF (weights first: the projection chains
        # need them before the first xT chunk lands) ----
        def load_w(dram, cols, tag):
            # one DMA: [D, cols] DRAM -> [128, DCH*cols] SBUF (d-chunks along
            # the free dim)
            t = persist.tile([128, DCH * cols], BF16, tag=tag)
            a = dram[:, :]
            src = bass.AP(tensor=a.tensor, offset=a.offset,
                          ap=[[cols, 128], [128 * cols, DCH], [1, cols]])
            nc.sync.dma_start(t[:].rearrange("p (c f) -> p c f", c=DCH), src)
            return t

        wq01 = load_w(wq01_d, 128, "wq01")
        xt = []
        for d in range(DCH):
            t = persist.tile([128, S], BF16, tag=f"xt{d}")
            xt.append(t)
        nc.sync.dma_start(xt[0][:], xT_d[0:128, :])
        wv = load_w(wv_d, 195, "wv")
        wk01 = load_w(wk01_d, 128, "wk01")
        wqk2 = load_w(wqk2_d, 128, "wqk2")
        for d in range(1, DCH):
            nc.sync.dma_start(xt[d][:], xT_d[d * 128:(d + 1) * 128, :])

        wo01 = persist.tile([128, D], BF16, tag="wo01")
        nc.sync.dma_start(wo01[:], wo01_d[:, :])
        # wo2 duplicated on both partition halves (rows 0-63 == rows 64-127)
        # so the paired zT2 out-proj matmuls have matching base partitions
        wo2 = persist.tile([128, D], BF16, tag="wo2")
        nc.sync.dma_start(wo2[:], wo2_d[:, :])

        def load_small(dram, shape, dt, tag):
            t = persist.tile(shape, dt, tag=tag)
            nc.sync.dma_start(t[:], dram[:, :])
            return t

        bq01 = load_small(bq01_d, [128, 1], F32, "bq01")
        bk01 = load_small(bk01_d, [128, 1], F32, "bk01")
        bqk2 = load_small(bqk2_d, [128, 1], F32, "bqk2")
        bv = load_small(bv_d, [128, 195], F32, "bv")
        tri = load_small(tri_d, [128, 128], BF16, "tri")

        # ---- persistent intermediates ----
        QT01 = persist.tile([128, S], BF16, tag="QT01")
        KT01 = persist.tile([128, S], BF16, tag="KT01")
        QKT2 = persist.tile([128, S], BF16, tag="QKT2")
        # QK2x rows 0-63 = K_h2 (low copy), rows 64-127 = Q_h2*s (high copy)
        QK2x = persist.tile([128, S], BF16, tag="QK2x")
        # V augmented with a ones column per head ([V_h | 1] x 3, 195 cols per
        # s-tile) so the PV matmul's 65th output row is the softmax denominator
        v_sb = persist.tile([128, NKT * 195], BF16, tag="v_sb")
        zT01 = persist.tile([128, S], BF16, tag="zT01")
        # zT2x rows 0-63 = z_h2, rows 64-127 = copy (for out-proj pairing)
        zT2x = persist.tile([128, S], BF16, tag="zT2x")

        # ---- QKV projections, n-outer so qb0 attention can start after the
        # first n-block of Q/K chains ----
        with tc.tile_pool(name="proj_ps", bufs=6, space="PSUM") as proj_pool, \
             tc.tile_pool(name="v_ps", bufs=2, space="PSUM") as v_pool:
            for n in range(S // 512):
                nsl = slice(n * 512, (n + 1) * 512)
                for ci, (w_s, bias_s, out_s) in enumerate(
                        ((wq01, bq01, QT01), (wk01, bk01, KT01),
                         (wqk2, bqk2, QKT2))):
                    ps = proj_pool.tile([128, 512], F32, tag="chain",
                                        name=f"ch{n}_{ci}")
                    for d in range(DCH):
                        nc.tensor.matmul(ps[:], w_s[:, d * 128:(d + 1) * 128],
                                         xt[d][:, nsl],
                                         start=(d == 0), stop=(d == DCH - 1))
                    nc.vector.tensor_scalar_add(out_s[:, nsl], ps[:], bias_s[:])

            for s_t in range(NKT):
                ps = v_pool.tile([128, 195], F32, tag="vps")
                for d in range(DCH):
                    nc.tensor.matmul(ps[:], xt[d][:, s_t * 128:(s_t + 1) * 128],
                                     wv[:, d * 195:(d + 1) * 195],
                                     start=(d == 0), stop=(d == DCH - 1))
                nc.vector.tensor_add(v_sb[:, s_t * 195:(s_t + 1) * 195], ps[:], bv[:])

        # partition-shift copies for h2 row-group pairing
        nc.sync.dma_start(QK2x[0:64, :], QKT2[64:128, :])    # K2 -> low
        nc.sync.dma_start(QK2x[64:128, :], QKT2[0:64, :])    # Q2 -> high

        # ---- attention + interleaved output projection ----
        # PSUM budget (8 banks): sT tag (scores [128,1024] / outproj
        # [128,768]) 2 slots = 4 banks; zts 4 x [65,512] = 4 banks.
        with tc.tile_pool(name="sT_ps", bufs=2, space="PSUM") as sT_pool, \
             tc.tile_pool(name="zT_ps", bufs=4, space="PSUM") as zT_pool, \
             tc.tile_pool(name="pt_sb", bufs=8) as pt_pool, \
             tc.tile_pool(name="rb_sb", bufs=3) as rb_pool, \
             tc.tile_pool(name="zs_sb", bufs=2) as zs_pool, \
             tc.tile_pool(name="out_sb", bufs=4) as out_pool, \
             tc.tile_pool(name="recip_dr", bufs=2, space="DRAM") as rdr_pool, \
             tc.tile_pool(name="recip_sb", bufs=2) as recip_pool:

            for qi in range(NQB):
                q0 = qi * QB
                J = 4 * qi + 4
                qsl = slice(q0, q0 + QB)

                def exp_mask(rr, st, name):
                    """exp+mask a [128, 1024] score tile whose halves are
                    k-tiles with diagonal offsets rr=(r0, r1); r<0 = fully
                    below diagonal (no masking)."""
                    pt = pt_pool.tile([128, 1024], BF16, tag="pt", name=name)
                    s0 = rr[0] * 128 if rr[0] >= 0 else 0
                    nc.scalar.activation(pt[:, s0:1024], st[:, s0:1024], AF.Exp)
                    for jj, r in enumerate(rr):
                        off = jj * 512
                        if r >= 0:
                            if r > 0:
                                nc.vector.memset(pt[:, off:off + r * 128], 0.0)
                            dsl = slice(off + r * 128, off + (r + 1) * 128)
                            nc.vector.tensor_mul(pt[:, dsl], pt[:, dsl], tri[:])
                    return pt

                zts = [zT_pool.tile([65, 512], F32, tag="zT", name=f"zt{i}")
                       for i in range(3)]

                def pv(kind, idx, pt):
                    if kind == "p":           # pair tile: heads 0,1 of k-tile idx
                        j = idx
                        for hv in range(2):
                            nc.tensor.matmul(
                                zts[hv][:],
                                v_sb[:, j * 195 + hv * 65:j * 195 + hv * 65 + 65],
                                pt[:, hv * 512:(hv + 1) * 512],
                                start=(j == 0), stop=(j == J - 1))
                    else:                     # h2 tile: k-tiles 2idx, 2idx+1
                        for jj in range(2):
                            j = 2 * idx + jj
                            nc.tensor.matmul(
                                zts[2][:],
                                v_sb[:, j * 195 + 130:j * 195 + 195],
                                pt[:, jj * 512:(jj + 1) * 512],
                                start=(j == 0), stop=(j == J - 1))

                # software pipeline with one round of skew: tile T's PV is
                # emitted after tile T+1's score matmuls, so the PE always
                # has independent work while ACT runs exp.
                rounds = [("p", j) for j in range(J)] + \
                         [("2", jp) for jp in range(J // 2)]
                pending = None

                for kind, idx in rounds:
                    st = sT_pool.tile([128, 1024], F32, tag="sT",
                                      name=f"st_{kind}{idx}")
                    if kind == "p":
                        j = idx
                        ksl = slice(j * 128, (j + 1) * 128)
                        # both halves of one k-tile: rows 0-63 (h0) and
                        # 64-127 (h1) -> row-group paired in the PE
                        nc.tensor.matmul(st[:, 0:512], KT01[0:64, ksl],
                                         QT01[0:64, qsl], start=True, stop=True)
                        nc.tensor.matmul(st[:, 512:1024], KT01[64:128, ksl],
                                         QT01[64:128, qsl], start=True, stop=True)
                        rr = (idx - 4 * qi, idx - 4 * qi)
                    else:
                        j0, j1 = 2 * idx, 2 * idx + 1
                        nc.tensor.matmul(st[:, 0:512],
                                         QK2x[0:64, j0 * 128:(j0 + 1) * 128],
                                         QKT2[0:64, qsl], start=True, stop=True)
                        nc.tensor.matmul(st[:, 512:1024],
                                         QKT2[64:128, j1 * 128:(j1 + 1) * 128],
                                         QK2x[64:128, qsl], start=True, stop=True)
                        rr = (j0 - 4 * qi, j1 - 4 * qi)

                    if pending is not None:
                        pv(*pending)
                    pending = (kind, idx, exp_mask(rr, st, f"pt{kind}{idx}"))
                pv(*pending)

                # normalize: zT_h = zt_h[0:64] * broadcast(1 / zt_h[64]).
                # The reciprocal of the 3x512 sums runs on a [128, 12]
                # reshape (via a DRAM bounce) — InstReciprocal costs ~6.5ns
                # per FREE element, so a [1, 512] layout would cost 3.3us.
                s3 = recip_pool.tile([1, 3 * 512], F32, tag="s3")
                for h in range(3):
                    nc.vector.tensor_copy(s3[:, h * 512:(h + 1) * 512],
                                          zts[h][64:65, :])
                dr1 = rdr_pool.tile([1, 3 * 512], F32, tag="dr1")
                nc.sync.dma_start(dr1[:], s3[:])
                rs = recip_pool.tile([128, 12], F32, tag="rs")
                nc.sync.dma_start(
                    rs[:], dr1[:].rearrange("o (p f) -> (o p) f", p=128))
                rr_t = recip_pool.tile([128, 12], F32, tag="rr")
                nc.vector.reciprocal(rr_t[:], rs[:])
                dr2 = rdr_pool.tile([1, 3 * 512], F32, tag="dr2")
                nc.sync.dma_start(
                    dr2[:].rearrange("o (p f) -> (o p) f", p=128), rr_t[:])
                rb = rb_pool.tile([64, 3 * 512], F32, tag="rb")
                for h in range(3):
                    nc.sync.dma_start(
                        rb[:, h * 512:(h + 1) * 512],
                        dr2[0:1, h * 512:(h + 1) * 512].broadcast_to([64, 512]))

                nc.vector.tensor_mul(zT01[0:64, qsl], zts[0][0:64, :],
                                     rb[:, 0:512])
                z1 = zs_pool.tile([64, 512], BF16, tag="z1")
                nc.vector.tensor_mul(z1[:], zts[1][0:64, :], rb[:, 512:1024])
                # head 1 lives on partitions 64-127 of zT01: DMA partition-shift
                nc.sync.dma_start(zT01[64:128, qsl], z1[:])
                z2 = zs_pool.tile([64, 512], BF16, tag="z2")
                nc.vector.tensor_mul(z2[:], zts[2][0:64, :], rb[:, 1024:1536])
                # z_h2 on both partition halves for out-proj row-group pairing
                nc.sync.dma_start(zT2x[0:64, qsl], z2[:])
                nc.sync.dma_start(zT2x[64:128, qsl], z2[:])

                # ---- output projection for this q-block's 4 row-tiles,
                # emitted as 2 pairs so the K=64 zT2 matmuls pair up ----
                for tp in range(2):
                    t0 = 4 * qi + 2 * tp
                    t1 = t0 + 1
                    sl0 = slice(t0 * 128, (t0 + 1) * 128)
                    sl1 = slice(t1 * 128, (t1 + 1) * 128)
                    ps0 = sT_pool.tile([128, D], F32, tag="sT", name=f"op{t0}")
                    ps1 = sT_pool.tile([128, D], F32, tag="sT", name=f"op{t1}")
                    for n0, nw in ((0, 512), (512, 256)):
                        nc.tensor.matmul(ps0[:, n0:n0 + nw], zT01[:, sl0],
                                         wo01[:, n0:n0 + nw],
                                         start=True, stop=False)
                        nc.tensor.matmul(ps1[:, n0:n0 + nw], zT01[:, sl1],
                                         wo01[:, n0:n0 + nw],
                                         start=True, stop=False)
                    for n0, nw in ((0, 512), (512, 256)):
                        # zT2 K=64: even tile on rows 0-63, odd on 64-127 ->
                        # row-group paired
                        nc.tensor.matmul(ps0[:, n0:n0 + nw], zT2x[0:64, sl0],
                                         wo2[0:64, n0:n0 + nw],
                                         start=False, stop=True)
                        nc.tensor.matmul(ps1[:, n0:n0 + nw],
                                         zT2x[64:128, sl1],
                                         wo2[64:128, n0:n0 + nw],
                                         start=False, stop=True)
                    for t, ps in ((t0, ps0), (t1, ps1)):
                        ob = out_pool.tile([128, D], F32, tag="ob")
                        if qi == NQB - 1:
                            # last block: ACT is done with exps, split halves
                            nc.vector.tensor_copy(ob[:, 0:384], ps[:, 0:384])
                            nc.scalar.copy(ob[:, 384:D], ps[:, 384:D])
                        else:
                            nc.vector.tensor_copy(ob[:], ps[:])
                        nc.sync.dma_start(out_d[t * 128:(t + 1) * 128, :], ob[:])

    nc.compile()
    return nc


def _get_nc():
    global _BUILT
    if _BUILT is None:
        _BUILT = build_nc()
    return _BUILT


def make_in_maps(inputs):
    x = np.asarray(inputs["normalized_resid_pre"], dtype=np.float32)
    W_Q = np.asarray(inputs["W_Q"], dtype=np.float32)
    W_K = np.asarray(inputs["W_K"], dtype=np.float32)
    W_V = np.asarray(inputs["W_V"], dtype=np.float32)
    W_O = np.asarray(inputs["W_O"], dtype=np.float32)
    b_Q = np.asarray(inputs["b_Q"], dtype=np.float32)
    b_K = np.asarray(inputs["b_K"], dtype=np.float32)
    b_V = np.asarray(inputs["b_V"], dtype=np.float32)
    sc = 1.0 / np.sqrt(np.float32(DH))

    in_maps = []
    for c in range(N_CORES):
        b = c // 4
        h = (c % 4) * 3
        hs = [h, h + 1, h + 2]
        m = {
            "xT": np.ascontiguousarray(x[b].T).astype(NPBF16),
            "wq01": np.concatenate([W_Q[hs[0]] * sc, W_Q[hs[1]] * sc],
                                   axis=1).astype(NPBF16),
            "wk01": np.concatenate([W_K[hs[0]], W_K[hs[1]]], axis=1).astype(NPBF16),
            "wqk2": np.concatenate([W_Q[hs[2]] * sc, W_K[hs[2]]],
                                   axis=1).astype(NPBF16),
            "wv": np.concatenate(
                sum(([W_V[hh], np.zeros((D, 1), np.float32)] for hh in hs), []),
                axis=1).astype(NPBF16),
            "wo01": np.concatenate([W_O[hs[0]], W_O[hs[1]]], axis=0).astype(NPBF16),
            "wo2": np.concatenate([W_O[hs[2]], W_O[hs[2]]], axis=0).astype(NPBF16),
            "bq01": (np.concatenate([b_Q[hs[0]], b_Q[hs[1]]]) * sc)[:, None]
                    .astype(np.float32),
            "bk01": np.concatenate([b_K[hs[0]], b_K[hs[1]]])[:, None]
                    .astype(np.float32),
            "bqk2": np.concatenate([b_Q[hs[2]] * sc, b_K[hs[2]]])[:, None]
                    .astype(np.float32),
            "bv": np.ascontiguousarray(np.broadcast_to(
                np.concatenate(
                    sum(([b_V[hh], np.ones(1, np.float32)] for hh in hs), [])),
                (128, 195))).astype(np.float32),
        }
        in_maps.append(m)
    return in_maps


def kernel(**inputs):
    global LAST_EXEC_NS, LAST_RESULT
    nc = _get_nc()
    in_maps = make_in_maps(inputs)
    b_O = np.asarray(inputs["b_O"], dtype=np.float32)

    res = run_bass_kernel_spmd(nc, in_maps, core_ids=list(range(N_CORES)),
                               trace=TRACE_ENABLED)
    LAST_EXEC_NS = res.exec_time_ns
    LAST_RESULT = res
    parts = [r["out_p"] for r in res.results]
    out0 = parts[0] + parts[1] + parts[2] + parts[3]
    out1 = parts[4] + parts[5] + parts[6] + parts[7]
    out = np.stack([out0, out1]) + b_O
    return out.astype(np.float32)


# revision 26
# speedup vs baseline: 1.1883x; 1.1883x over previous
"""Multi-head causal attention (dense transformer block) on 8 TRN2 NeuronCores.

Problem: B=2, S=2048, D_MODEL=768, H=12 heads, D_HEAD=64, fp32 I/O.

Sharding: 24 (batch, head) units over 8 cores -> 3 heads x 1 batch per core.
Cores 0-3 handle batch 0 (heads 0-2, 3-5, 6-8, 9-11), cores 4-7 batch 1.
Each core computes its heads' contribution to out[b] = sum_h z_h @ W_O[h];
the host sums the 4 partials per batch and adds b_O.

Per-core dataflow (all matmuls bf16 -> fp32 PSUM):
  - x[b]^T staged in SBUF as 6 chunks [128, 2048].
  - QT/KT in "head-transposed" layout [64, S] packed 2-per-tile:
      QT01 = [Q_h0*s | Q_h1*s]^T x  (scale 1/sqrt(64) folded into W_Q/b_Q)
      KT01 = [K_h0 | K_h1]^T x, QKT2 = [Q_h2*s | K_h2]^T x
    QK2x = [K_h2 (rows 0-63) | Q_h2*s (rows 64-127)] partition-shift copies so
    h2 score matmuls for consecutive k-tiles run on disjoint row halves.
  - scores TRANSPOSED: sT[k, q] = KT^T-slice x QT (K=64 contraction). Each
    PSUM score tile [128, 1024] holds BOTH halves of a row-group pair
    ([h0_j | h1_j], or [h2_j | h2_j+1]) so the two K=64 matmuls are released
    together and execute CONCURRENTLY in the PE array (row-group tiling,
    ~2x measured).
  - exp via ScalarE (one instr per [128, 1024] tile), causal mask via
    memset + triangular-mask multiply on diagonal 128x128 tiles (DVE).
  - zT[h] = sum_j [V_h[j] | 1]^T x PT[j]  (unnormalized; the ones column
    makes output row 64 the softmax denominator), normalized as
    zT * broadcast(1/sums) when copying PSUM->SBUF.
  - output projection interleaved per q-block (overlaps the next block's
    ACT-bound attention): out[q, :] = zT01^T-slice @ WO01 + zT2^T @ WO2,
    with the K=64 zT2 matmuls of adjacent q-tiles row-group paired via a
    high-half copy of zT2.
"""

import numpy as np
import ml_dtypes
from contextlib import ExitStack

import concourse.bass as bass
import concourse.mybir as mybir
import concourse.tile as tile
from concourse import bacc
from concourse.bass_utils import run_bass_kernel_spmd

BF16 = mybir.dt.bfloat16
F32 = mybir.dt.float32
AF = mybir.ActivationFunctionType
NPBF16 = ml_dtypes.bfloat16

B, S, D, H, DH = 2, 2048, 768, 12, 64
N_CORES = 8
DCH = D // 128          # 6 d_model chunks
NKT = S // 128          # 16 k tiles
QB = 512                # q block width
NQB = S // QB           # 4 q blocks

TRACE_ENABLED = False
LAST_EXEC_NS = None
LAST_RESULT = None
_BUILT = None


def build_nc():
    nc = bacc.Bacc("TRN2", target_bir_lowering=False, debug=False)

    xT_d = nc.dram_tensor("xT", [D, S], BF16, kind="ExternalInput")
    wq01_d = nc.dram_tensor("wq01", [D, 128], BF16, kind="ExternalInput")
    wk01_d = nc.dram_tensor("wk01", [D, 128], BF16, kind="ExternalInput")
    wqk2_d = nc.dram_tensor("wqk2", [D, 128], BF16, kind="ExternalInput")
    wv_d = nc.dram_tensor("wv", [D, 192], BF16, kind="ExternalInput")
    wo01_d = nc.dram_tensor("wo01", [128, D], BF16, kind="ExternalInput")
    wo2_d = nc.dram_tensor("wo2", [128, D], BF16, kind="ExternalInput")
    bq01_d = nc.dram_tensor("bq01", [128, 1], F32, kind="ExternalInput")
    bk01_d = nc.dram_tensor("bk01", [128, 1], F32, kind="ExternalInput")
    bqk2_d = nc.dram_tensor("bqk2", [128, 1], F32, kind="ExternalInput")
    bv_d = nc.dram_tensor("bv", [128, 192], F32, kind="ExternalInput")
    out_d = nc.dram_tensor("out_p", [S, D], F32, kind="ExternalOutput")

    tri_np = np.triu(np.ones((128, 128), np.float32)).astype(NPBF16)
    tri_d = nc.inline_tensor(tri_np, "tri")

    with tile.TileContext(nc) as tc, ExitStack() as ctx:
        persist = ctx.enter_context(tc.tile_pool(name="persist", bufs=1))

        # ---- HAM warm-up: dummy matmuls spanning the input-DMA wait, so
        # the PE clock is ramping while xT lands ----
        with tc.tile_pool(name="warm_ps", bufs=1, space="PSUM") as warm_pool:
            wz = persist.tile([128, 128], BF16, tag="wz")
            nc.vector.memset(wz[:], 0.0)
            wps = warm_pool.tile([128, 128], F32, tag="warm")
            for _ in range(36):
                nc.tensor.matmul(wps[:], wz[:], wz[:], start=True, stop=True)

        # ---- stage inputs in SBUF (weights first: the projection chains
        # need them before the first xT chunk lands) ----
        def load_w(dram, cols, tag):
            # one DMA: [D, cols] DRAM -> [128, DCH*cols] SBUF (d-chunks along
            # the free dim)
            t = persist.tile([128, DCH * cols], BF16, tag=tag)
            a = dram[:, :]
            src = bass.AP(tensor=a.tensor, offset=a.offset,
                          ap=[[cols, 128], [128 * cols, DCH], [1, cols]])
            nc.sync.dma_start(t[:].rearrange("p (c f) -> p c f", c=DCH), src)
            return t

        wq01 = load_w(wq01_d, 128, "wq01")
        xt = []
        for d in range(DCH):
            t = persist.tile([128, S], BF16, tag=f"xt{d}")
            xt.append(t)
        nc.sync.dma_start(xt[0][:], xT_d[0:128, :])
        wv = load_w(wv_d, 192, "wv")
        wk01 = load_w(wk01_d, 128, "wk01")
        wqk2 = load_w(wqk2_d, 128, "wqk2")
        for d in range(1, DCH):
            nc.sync.dma_start(xt[d][:], xT_d[d * 128:(d + 1) * 128, :])

        wo01 = persist.tile([128, D], BF16, tag="wo01")
        nc.sync.dma_start(wo01[:], wo01_d[:, :])
        # wo2 duplicated on both partition halves (rows 0-63 == rows 64-127)
        # so the paired zT2 out-proj matmuls have matching base partitions
        wo2 = persist.tile([128, D], BF16, tag="wo2")
        nc.sync.dma_start(wo2[:], wo2_d[:, :])

        def load_small(dram, shape, dt, tag):
            t = persist.tile(shape, dt, tag=tag)
            nc.sync.dma_start(t[:], dram[:, :])
            return t

        bq01 = load_small(bq01_d, [128, 1], F32, "bq01")
        bk01 = load_small(bk01_d, [128, 1], F32, "bk01")
        bqk2 = load_small(bqk2_d, [128, 1], F32, "bqk2")
        bv = load_small(bv_d, [128, 192], F32, "bv")
        tri = load_small(tri_d, [128, 128], BF16, "tri")

        # ---- persistent intermediates ----
        QT01 = persist.tile([128, S], BF16, tag="QT01")
        KT01 = persist.tile([128, S], BF16, tag="KT01")
        QKT2 = persist.tile([128, S], BF16, tag="QKT2")
        # QK2x rows 0-63 = K_h2 (low copy), rows 64-127 = Q_h2*s (high copy)
        QK2x = persist.tile([128, S], BF16, tag="QK2x")
        # V augmented with 64 ones columns per head ([V_h | ones64] x 3,
        # 384 cols per s-tile) so the PV matmul's output rows 64-127 are the
        # softmax denominator REPLICATED across 64 partitions — the
        # reciprocal+normalize then needs only one partition-shift DMA.
        # The ones blocks are constant: memset once here.
        v_sb = persist.tile([128, NKT * 384], BF16, tag="v_sb")
        nc.vector.memset(
            v_sb[:].rearrange("p (t h c) -> p t h c", t=NKT, h=3)[:, :, :, 64:128],
            1.0)
        zT01 = persist.tile([128, S], BF16, tag="zT01")
        # zT2x rows 0-63 = z_h2, rows 64-127 = copy (for out-proj pairing)
        zT2x = persist.tile([128, S], BF16, tag="zT2x")

        # ---- QKV projections, n-outer so qb0 attention can start after the
        # first n-block of Q/K chains ----
        with tc.tile_pool(name="proj_ps", bufs=6, space="PSUM") as proj_pool, \
             tc.tile_pool(name="v_ps", bufs=2, space="PSUM") as v_pool:
            for n in range(S // 512):
                nsl = slice(n * 512, (n + 1) * 512)
                for ci, (w_s, bias_s, out_s) in enumerate(
                        ((wq01, bq01, QT01), (wk01, bk01, KT01),
                         (wqk2, bqk2, QKT2))):
                    ps = proj_pool.tile([128, 512], F32, tag="chain",
                                        name=f"ch{n}_{ci}")
                    for d in range(DCH):
                        nc.tensor.matmul(ps[:], w_s[:, d * 128:(d + 1) * 128],
                                         xt[d][:, nsl],
                                         start=(d == 0), stop=(d == DCH - 1))
                    nc.vector.tensor_scalar_add(out_s[:, nsl], ps[:], bias_s[:])

            for s_t in range(NKT):
                ps = v_pool.tile([128, 192], F32, tag="vps")
                for d in range(DCH):
                    nc.tensor.matmul(ps[:], xt[d][:, s_t * 128:(s_t + 1) * 128],
                                     wv[:, d * 192:(d + 1) * 192],
                                     start=(d == 0), stop=(d == DCH - 1))
                vdst = v_sb[:, s_t * 384:(s_t + 1) * 384] \
                    .rearrange("p (h c) -> p h c", h=3)[:, :, 0:64]
                nc.vector.tensor_add(
                    vdst, ps[:].rearrange("p (h c) -> p h c", h=3),
                    bv[:].rearrange("p (h c) -> p h c", h=3))

        # partition-shift copies for h2 row-group pairing
        nc.sync.dma_start(QK2x[0:64, :], QKT2[64:128, :])    # K2 -> low
        nc.sync.dma_start(QK2x[64:128, :], QKT2[0:64, :])    # Q2 -> high

        # ---- attention + interleaved output projection ----
        # PSUM budget (8 banks): sT tag (scores [128,1024] / outproj
        # [128,768]) 2 slots = 4 banks; zts 4 x [65,512] = 4 banks.
        with tc.tile_pool(name="sT_ps", bufs=2, space="PSUM") as sT_pool, \
             tc.tile_pool(name="zT_ps", bufs=4, space="PSUM") as zT_pool, \
             tc.tile_pool(name="pt_sb", bufs=8) as pt_pool, \
             tc.tile_pool(name="rb_sb", bufs=3) as rb_pool, \
             tc.tile_pool(name="zs_sb", bufs=2) as zs_pool, \
             tc.tile_pool(name="out_sb", bufs=4) as out_pool, \
             tc.tile_pool(name="recip_dr", bufs=2, space="DRAM") as rdr_pool, \
             tc.tile_pool(name="recip_sb", bufs=2) as recip_pool:

            deferred_op = None

            def make_op_emitter(qi):
                """Emit output projection for q-block qi's 2 tile-pairs."""
                def emit(tp):
                    t0 = 4 * qi + 2 * tp
                    t1 = t0 + 1
                    sl0 = slice(t0 * 128, (t0 + 1) * 128)
                    sl1 = slice(t1 * 128, (t1 + 1) * 128)
                    ps0 = sT_pool.tile([128, D], F32, tag="sT", name=f"op{t0}")
                    ps1 = sT_pool.tile([128, D], F32, tag="sT", name=f"op{t1}")
                    for n0, nw in ((0, 512), (512, 256)):
                        nc.tensor.matmul(ps0[:, n0:n0 + nw], zT01[:, sl0],
                                         wo01[:, n0:n0 + nw],
                                         start=True, stop=False)
                        nc.tensor.matmul(ps1[:, n0:n0 + nw], zT01[:, sl1],
                                         wo01[:, n0:n0 + nw],
                                         start=True, stop=False)
                    for n0, nw in ((0, 512), (512, 256)):
                        # zT2 K=64: even tile on rows 0-63, odd on 64-127 ->
                        # row-group paired
                        nc.tensor.matmul(ps0[:, n0:n0 + nw], zT2x[0:64, sl0],
                                         wo2[0:64, n0:n0 + nw],
                                         start=False, stop=True)
                        nc.tensor.matmul(ps1[:, n0:n0 + nw],
                                         zT2x[64:128, sl1],
                                         wo2[64:128, n0:n0 + nw],
                                         start=False, stop=True)
                    for t, ps in ((t0, ps0), (t1, ps1)):
                        ob = out_pool.tile([128, D], F32, tag="ob")
                        if qi == NQB - 1:
                            # last block: ACT is done with exps, split halves
                            nc.vector.tensor_copy(ob[:, 0:384], ps[:, 0:384])
                            nc.scalar.copy(ob[:, 384:D], ps[:, 384:D])
                        else:
                            nc.vector.tensor_copy(ob[:], ps[:])
                        nc.sync.dma_start(out_d[t * 128:(t + 1) * 128, :], ob[:])
                return emit

            for qi in range(NQB):
                q0 = qi * QB
                J = 4 * qi + 4
                qsl = slice(q0, q0 + QB)

                def exp_mask(rr, st, name):
                    """exp+mask a [128, 1024] score tile whose halves are
                    k-tiles with diagonal offsets rr=(r0, r1); r<0 = fully
                    below diagonal (no masking). Memsets go to the otherwise
                    idle GpSimd engine to keep DVE load down."""
                    pt = pt_pool.tile([128, 1024], BF16, tag="pt", name=name)
                    s0 = rr[0] * 128 if rr[0] >= 0 else 0
                    nc.scalar.activation(pt[:, s0:1024], st[:, s0:1024], AF.Exp)
                    for jj, r in enumerate(rr):
                        off = jj * 512
                        if r >= 0:
                            if r > 0:
                                nc.gpsimd.memset(pt[:, off:off + r * 128], 0.0)
                            dsl = slice(off + r * 128, off + (r + 1) * 128)
                            nc.vector.tensor_mul(pt[:, dsl], pt[:, dsl], tri[:])
                    return pt

                zts = [zT_pool.tile([128, 512], F32, tag="zT", name=f"zt{i}")
                       for i in range(3)]

                def pv(kind, idx, pt):
                    if kind == "p":           # pair tile: heads 0,1 of k-tile idx
                        j = idx
                        for hv in range(2):
                            nc.tensor.matmul(
                                zts[hv][:],
                                v_sb[:, j * 384 + hv * 128:j * 384 + (hv + 1) * 128],
                                pt[:, hv * 512:(hv + 1) * 512],
                                start=(j == 0), stop=(j == J - 1))
                    else:                     # h2 tile: k-tiles 2idx, 2idx+1
                        for jj in range(2):
                            j = 2 * idx + jj
                            nc.tensor.matmul(
                                zts[2][:],
                                v_sb[:, j * 384 + 256:j * 384 + 384],
                                pt[:, jj * 512:(jj + 1) * 512],
                                start=(j == 0), stop=(j == J - 1))

                # software pipeline with one round of skew: tile T's PV is
                # emitted after tile T+1's score matmuls, so the PE always
                # has independent work while ACT runs exp.
                rounds = [("p", j) for j in range(J)] + \
                         [("2", jp) for jp in range(J // 2)]
                pending = None

                i1 = len(rounds) // 2
                i2 = i1 + 3
                for ridx, (kind, idx) in enumerate(rounds):
                    # inject the previous q-block's output projection away
                    # from the block boundary (its slot WARs then resolve via
                    # fast exp releases, and its zT deps — the normalize
                    # bounce — have had ~i1 tiles of time to land)
                    if deferred_op is not None and ridx in (i1, i2):
                        deferred_op(0 if ridx == i1 else 1)
                        if ridx == i2:
                            deferred_op = None
                    st = sT_pool.tile([128, 1024], F32, tag="sT",
                                      name=f"st_{kind}{idx}")
                    if kind == "p":
                        j = idx
                        ksl = slice(j * 128, (j + 1) * 128)
                        # both halves of one k-tile: rows 0-63 (h0) and
                        # 64-127 (h1) -> row-group paired in the PE
                        nc.tensor.matmul(st[:, 0:512], KT01[0:64, ksl],
                                         QT01[0:64, qsl], start=True, stop=True)
                        nc.tensor.matmul(st[:, 512:1024], KT01[64:128, ksl],
                                         QT01[64:128, qsl], start=True, stop=True)
                        rr = (idx - 4 * qi, idx - 4 * qi)
                    else:
                        j0, j1 = 2 * idx, 2 * idx + 1
                        nc.tensor.matmul(st[:, 0:512],
                                         QK2x[0:64, j0 * 128:(j0 + 1) * 128],
                                         QKT2[0:64, qsl], start=True, stop=True)
                        nc.tensor.matmul(st[:, 512:1024],
                                         QKT2[64:128, j1 * 128:(j1 + 1) * 128],
                                         QK2x[64:128, qsl], start=True, stop=True)
                        rr = (j0 - 4 * qi, j1 - 4 * qi)

                    if pending is not None:
                        pv(*pending)
                    pending = (kind, idx, exp_mask(rr, st, f"pt{kind}{idx}"))
                pv(*pending)

                # normalize: zT_h = zt_h[0:64] * broadcast(1 / zt_h[64]).
                # The reciprocal of the 3x512 sums runs on a [128, 12]
                # reshape (via a DRAM bounce) — InstReciprocal costs ~6.5ns
                # per FREE element, so a [64, 512] layout would cost 3.3us.
                # The bounce latency (~8us) hides under the next q-block's
                # score/exp stream (sT rotation is independent of it).
                s3 = recip_pool.tile([1, 3 * 512], F32, tag="s3")
                for h in range(3):
                    nc.vector.tensor_copy(s3[:, h * 512:(h + 1) * 512],
                                          zts[h][64:65, :])
                dr1 = rdr_pool.tile([1, 3 * 512], F32, tag="dr1")
                nc.sync.dma_start(dr1[:], s3[:])
                rs = recip_pool.tile([128, 12], F32, tag="rs")
                nc.sync.dma_start(
                    rs[:], dr1[:].rearrange("o (p f) -> (o p) f", p=128))
                rr_t = recip_pool.tile([128, 12], F32, tag="rr")
                nc.vector.reciprocal(rr_t[:], rs[:])
                dr2 = rdr_pool.tile([1, 3 * 512], F32, tag="dr2")
                nc.sync.dma_start(
                    dr2[:].rearrange("o (p f) -> (o p) f", p=128), rr_t[:])
                rb = rb_pool.tile([64, 3 * 512], F32, tag="rb")
                for h in range(3):
                    nc.sync.dma_start(
                        rb[:, h * 512:(h + 1) * 512],
                        dr2[0:1, h * 512:(h + 1) * 512].broadcast_to([64, 512]))

                nc.vector.tensor_mul(zT01[0:64, qsl], zts[0][0:64, :],
                                     rb[:, 0:512])
                z1 = zs_pool.tile([64, 512], BF16, tag="z1")
                nc.vector.tensor_mul(z1[:], zts[1][0:64, :], rb[:, 512:1024])
                # head 1 lives on partitions 64-127 of zT01: DMA partition-shift
                nc.sync.dma_start(zT01[64:128, qsl], z1[:])
                z2 = zs_pool.tile([64, 512], BF16, tag="z2")
                nc.vector.tensor_mul(z2[:], zts[2][0:64, :], rb[:, 1024:1536])
                # z_h2 on both partition halves for out-proj row-group pairing
                nc.sync.dma_start(zT2x[0:64, qsl], z2[:])
                nc.sync.dma_start(zT2x[64:128, qsl], z2[:])

                deferred_op = make_op_emitter(qi)

            # last q-block's output projection (ACT is free: split copies)
            deferred_op(0)
            deferred_op(1)

    nc.compile()
    return nc


def _get_nc():
    global _BUILT
    if _BUILT is None:
        _BUILT = build_nc()
    return _BUILT


def make_in_maps(inputs):
    x = np.asarray(inputs["normalized_resid_pre"], dtype=np.float32)
    W_Q = np.asarray(inputs["W_Q"], dtype=np.float32)
    W_K = np.asarray(inputs["W_K"], dtype=np.float32)
    W_V = np.asarray(inputs["W_V"], dtype=np.float32)
    W_O = np.asarray(inputs["W_O"], dtype=np.float32)
    b_Q = np.asarray(inputs["b_Q"], dtype=np.float32)
    b_K = np.asarray(inputs["b_K"], dtype=np.float32)
    b_V = np.asarray(inputs["b_V"], dtype=np.float32)
    sc = 1.0 / np.sqrt(np.float32(DH))

    in_maps = []
    for c in range(N_CORES):
        b = c // 4
        h = (c % 4) * 3
        hs = [h, h + 1, h + 2]
        m = {
            "xT": np.ascontiguousarray(x[b].T).astype(NPBF16),
            "wq01": np.concatenate([W_Q[hs[0]] * sc, W_Q[hs[1]] * sc],
                                   axis=1).astype(NPBF16),
            "wk01": np.concatenate([W_K[hs[0]], W_K[hs[1]]], axis=1).astype(NPBF16),
            "wqk2": np.concatenate([W_Q[hs[2]] * sc, W_K[hs[2]]],
                                   axis=1).astype(NPBF16),
            "wv": np.concatenate([W_V[hh] for hh in hs],
                                 axis=1).astype(NPBF16),
            "wo01": np.concatenate([W_O[hs[0]], W_O[hs[1]]], axis=0).astype(NPBF16),
            "wo2": np.concatenate([W_O[hs[2]], W_O[hs[2]]], axis=0).astype(NPBF16),
            "bq01": (np.concatenate([b_Q[hs[0]], b_Q[hs[1]]]) * sc)[:, None]
                    .astype(np.float32),
            "bk01": np.concatenate([b_K[hs[0]], b_K[hs[1]]])[:, None]
                    .astype(np.float32),
            "bqk2": np.concatenate([b_Q[hs[2]] * sc, b_K[hs[2]]])[:, None]
                    .astype(np.float32),
            "bv": np.ascontiguousarray(np.broadcast_to(
                np.concatenate([b_V[hh] for hh in hs]),
                (128, 192))).astype(np.float32),
        }
        in_maps.append(m)
    return in_maps


def kernel(**inputs):
    global LAST_EXEC_NS, LAST_RESULT
    nc = _get_nc()
    in_maps = make_in_maps(inputs)
    b_O = np.asarray(inputs["b_O"], dtype=np.float32)

    res = run_bass_kernel_spmd(nc, in_maps, core_ids=list(range(N_CORES)),
                               trace=TRACE_ENABLED)
    LAST_EXEC_NS = res.exec_time_ns
    LAST_RESULT = res
    parts = [r["out_p"] for r in res.results]
    out0 = parts[0] + parts[1] + parts[2] + parts[3]
    out1 = parts[4] + parts[5] + parts[6] + parts[7]
    out = np.stack([out0, out1]) + b_O
    return out.astype(np.float32)


# revision 30
# speedup vs baseline: 1.2432x; 1.0463x over previous
"""Multi-head causal attention (dense transformer block) on 8 TRN2 NeuronCores.

Problem: B=2, S=2048, D_MODEL=768, H=12 heads, D_HEAD=64, fp32 I/O.

Sharding: 24 (batch, head) units over 8 cores -> 3 heads x 1 batch per core.
Cores 0-3 handle batch 0 (heads 0-2, 3-5, 6-8, 9-11), cores 4-7 batch 1.
Each core computes its heads' contribution to out[b] = sum_h z_h @ W_O[h];
the host sums the 4 partials per batch and adds b_O.

Per-core dataflow (all matmuls bf16 -> fp32 PSUM):
  - x[b]^T staged in SBUF as 6 chunks [128, 2048].
  - QT/KT in "head-transposed" layout [64, S] packed 2-per-tile:
      QT01 = [Q_h0*s | Q_h1*s]^T x  (scale 1/sqrt(64) folded into W_Q/b_Q)
      KT01 = [K_h0 | K_h1]^T x, QKT2 = [Q_h2*s | K_h2]^T x
    QK2x = [K_h2 (rows 0-63) | Q_h2*s (rows 64-127)] partition-shift copies so
    h2 score matmuls for consecutive k-tiles run on disjoint row halves.
  - scores TRANSPOSED: sT[k, q] = KT^T-slice x QT (K=64 contraction). Each
    PSUM score tile [128, 1024] holds BOTH halves of a row-group pair
    ([h0_j | h1_j], or [h2_j | h2_j+1]) so the two K=64 matmuls are released
    together and execute CONCURRENTLY in the PE array (row-group tiling,
    ~2x measured).
  - exp via ScalarE (one instr per [128, 1024] tile), causal mask via
    memset + triangular-mask multiply on diagonal 128x128 tiles (DVE).
  - zT[h] = sum_j [V_h[j] | 1]^T x PT[j]  (unnormalized; the ones column
    makes output row 64 the softmax denominator), normalized as
    zT * broadcast(1/sums) when copying PSUM->SBUF.
  - output projection interleaved per q-block (overlaps the next block's
    ACT-bound attention): out[q, :] = zT01^T-slice @ WO01 + zT2^T @ WO2,
    with the K=64 zT2 matmuls of adjacent q-tiles row-group paired via a
    high-half copy of zT2.
"""

import numpy as np
import ml_dtypes
from contextlib import ExitStack

import concourse.bass as bass
import concourse.mybir as mybir
import concourse.tile as tile
from concourse import bacc
from concourse.bass_utils import run_bass_kernel_spmd

BF16 = mybir.dt.bfloat16
F32 = mybir.dt.float32
AF = mybir.ActivationFunctionType
NPBF16 = ml_dtypes.bfloat16

B, S, D, H, DH = 2, 2048, 768, 12, 64
N_CORES = 8
DCH = D // 128          # 6 d_model chunks
NKT = S // 128          # 16 k tiles
QB = 512                # q block width
NQB = S // QB           # 4 q blocks

TRACE_ENABLED = False
LAST_EXEC_NS = None
LAST_RESULT = None
_BUILT = None


def build_nc():
    nc = bacc.Bacc("TRN2", target_bir_lowering=False, debug=False)

    xT_d = nc.dram_tensor("xT", [D, S], BF16, kind="ExternalInput")
    wq01_d = nc.dram_tensor("wq01", [D, 128], BF16, kind="ExternalInput")
    wk01_d = nc.dram_tensor("wk01", [D, 128], BF16, kind="ExternalInput")
    wqk2_d = nc.dram_tensor("wqk2", [D, 128], BF16, kind="ExternalInput")
    wv_d = nc.dram_tensor("wv", [D, 192], BF16, kind="ExternalInput")
    wo01_d = nc.dram_tensor("wo01", [128, D], BF16, kind="ExternalInput")
    wo2_d = nc.dram_tensor("wo2", [128, D], BF16, kind="ExternalInput")
    bq01_d = nc.dram_tensor("bq01", [128, 1], F32, kind="ExternalInput")
    bk01_d = nc.dram_tensor("bk01", [128, 1], F32, kind="ExternalInput")
    bqk2_d = nc.dram_tensor("bqk2", [128, 1], F32, kind="ExternalInput")
    bv_d = nc.dram_tensor("bv", [128, 192], F32, kind="ExternalInput")
    out_d = nc.dram_tensor("out_p", [S, D], F32, kind="ExternalOutput")

    tri_np = np.triu(np.ones((128, 128), np.float32)).astype(NPBF16)
    tri_d = nc.inline_tensor(tri_np, "tri")

    with tile.TileContext(nc) as tc, ExitStack() as ctx:
        persist = ctx.enter_context(tc.tile_pool(name="persist", bufs=1))

        # ---- HAM warm-up: dummy matmuls spanning the input-DMA wait, so
        # the PE clock is ramping while xT lands ----
        with tc.tile_pool(name="warm_ps", bufs=1, space="PSUM") as warm_pool:
            wz = persist.tile([128, 128], BF16, tag="wz")
            nc.vector.memset(wz[:], 0.0)
            wps = warm_pool.tile([128, 128], F32, tag="warm")
            for _ in range(36):
                nc.tensor.matmul(wps[:], wz[:], wz[:], start=True, stop=True)

        # ---- stage inputs in SBUF (weights first: the projection chains
        # need them before the first xT chunk lands) ----
        def load_w(dram, cols, tag):
            # one DMA: [D, cols] DRAM -> [128, DCH*cols] SBUF (d-chunks along
            # the free dim)
            t = persist.tile([128, DCH * cols], BF16, tag=tag)
            a = dram[:, :]
            src = bass.AP(tensor=a.tensor, offset=a.offset,
                          ap=[[cols, 128], [128 * cols, DCH], [1, cols]])
            nc.sync.dma_start(t[:].rearrange("p (c f) -> p c f", c=DCH), src)
            return t

        wq01 = load_w(wq01_d, 128, "wq01")
        xt = []
        for d in range(DCH):
            t = persist.tile([128, S], BF16, tag=f"xt{d}")
            xt.append(t)
        nc.sync.dma_start(xt[0][:], xT_d[0:128, :])
        wv = load_w(wv_d, 192, "wv")
        wk01 = load_w(wk01_d, 128, "wk01")
        wqk2 = load_w(wqk2_d, 128, "wqk2")
        for d in range(1, DCH):
            nc.sync.dma_start(xt[d][:], xT_d[d * 128:(d + 1) * 128, :])

        wo01 = persist.tile([128, D], BF16, tag="wo01")
        nc.sync.dma_start(wo01[:], wo01_d[:, :])
        # wo2 duplicated on both partition halves (rows 0-63 == rows 64-127)
        # so the paired zT2 out-proj matmuls have matching base partitions
        wo2 = persist.tile([128, D], BF16, tag="wo2")
        nc.sync.dma_start(wo2[:], wo2_d[:, :])

        def load_small(dram, shape, dt, tag):
            t = persist.tile(shape, dt, tag=tag)
            nc.sync.dma_start(t[:], dram[:, :])
            return t

        bq01 = load_small(bq01_d, [128, 1], F32, "bq01")
        bk01 = load_small(bk01_d, [128, 1], F32, "bk01")
        bqk2 = load_small(bqk2_d, [128, 1], F32, "bqk2")
        bv = load_small(bv_d, [128, 192], F32, "bv")
        tri = load_small(tri_d, [128, 128], BF16, "tri")

        # ---- persistent intermediates ----
        QT01 = persist.tile([128, S], BF16, tag="QT01")
        KT01 = persist.tile([128, S], BF16, tag="KT01")
        QKT2 = persist.tile([128, S], BF16, tag="QKT2")
        # QK2x rows 0-63 = K_h2 (low copy), rows 64-127 = Q_h2*s (high copy)
        QK2x = persist.tile([128, S], BF16, tag="QK2x")
        # V augmented with 64 ones columns per head ([V_h | ones64] x 3,
        # 384 cols per s-tile) so the PV matmul's output rows 64-127 are the
        # softmax denominator REPLICATED across 64 partitions — the
        # reciprocal+normalize then needs only one partition-shift DMA.
        # The ones blocks are constant: memset once here.
        v_sb = persist.tile([128, NKT * 384], BF16, tag="v_sb")
        nc.vector.memset(
            v_sb[:].rearrange("p (t h c) -> p t h c", t=NKT, h=3)[:, :, :, 64:128],
            1.0)
        zT01 = persist.tile([128, S], BF16, tag="zT01")
        # zT2x rows 0-63 = z_h2, rows 64-127 = copy (for out-proj pairing)
        zT2x = persist.tile([128, S], BF16, tag="zT2x")

        # ---- QKV projections, n-outer so qb0 attention can start after the
        # first n-block of Q/K chains ----
        with tc.tile_pool(name="proj_ps", bufs=6, space="PSUM") as proj_pool, \
             tc.tile_pool(name="v_ps", bufs=2, space="PSUM") as v_pool:
            for n in range(S // 512):
                nsl = slice(n * 512, (n + 1) * 512)
                for ci, (w_s, bias_s, out_s) in enumerate(
                        ((wq01, bq01, QT01), (wk01, bk01, KT01),
                         (wqk2, bqk2, QKT2))):
                    ps = proj_pool.tile([128, 512], F32, tag="chain",
                                        name=f"ch{n}_{ci}")
                    for d in range(DCH):
                        nc.tensor.matmul(ps[:], w_s[:, d * 128:(d + 1) * 128],
                                         xt[d][:, nsl],
                                         start=(d == 0), stop=(d == DCH - 1))
                    nc.vector.tensor_scalar_add(out_s[:, nsl], ps[:], bias_s[:])

            for s_t in range(NKT):
                ps = v_pool.tile([128, 192], F32, tag="vps")
                for d in range(DCH):
                    nc.tensor.matmul(ps[:], xt[d][:, s_t * 128:(s_t + 1) * 128],
                                     wv[:, d * 192:(d + 1) * 192],
                                     start=(d == 0), stop=(d == DCH - 1))
                vdst = v_sb[:, s_t * 384:(s_t + 1) * 384] \
                    .rearrange("p (h c) -> p h c", h=3)[:, :, 0:64]
                nc.vector.tensor_add(
                    vdst, ps[:].rearrange("p (h c) -> p h c", h=3),
                    bv[:].rearrange("p (h c) -> p h c", h=3))

        # partition-shift copies for h2 row-group pairing
        nc.sync.dma_start(QK2x[0:64, :], QKT2[64:128, :])    # K2 -> low
        nc.sync.dma_start(QK2x[64:128, :], QKT2[0:64, :])    # Q2 -> high

        # ---- attention + interleaved output projection ----
        # PSUM budget (8 banks): sT tag (scores [128,1024] / outproj
        # [128,768]) 2 slots = 4 banks; zts 4 x [65,512] = 4 banks.
        with tc.tile_pool(name="sT_ps", bufs=2, space="PSUM") as sT_pool, \
             tc.tile_pool(name="zT_ps", bufs=4, space="PSUM") as zT_pool, \
             tc.tile_pool(name="pt_sb", bufs=16) as pt_pool, \
             tc.tile_pool(name="rb_sb", bufs=3) as rb_pool, \
             tc.tile_pool(name="zs_sb", bufs=2) as zs_pool, \
             tc.tile_pool(name="out_sb", bufs=4) as out_pool, \
             tc.tile_pool(name="recip_dr", bufs=2, space="DRAM") as rdr_pool, \
             tc.tile_pool(name="recip_sb", bufs=2) as recip_pool:

            deferred_op = None

            def make_op_emitter(qi):
                """Emit output projection for q-block qi's 2 tile-pairs."""
                def emit(tp):
                    t0 = 4 * qi + 2 * tp
                    t1 = t0 + 1
                    sl0 = slice(t0 * 128, (t0 + 1) * 128)
                    sl1 = slice(t1 * 128, (t1 + 1) * 128)
                    ps0 = sT_pool.tile([128, D], F32, tag="sT", name=f"op{t0}")
                    ps1 = sT_pool.tile([128, D], F32, tag="sT", name=f"op{t1}")
                    for n0, nw in ((0, 512), (512, 256)):
                        nc.tensor.matmul(ps0[:, n0:n0 + nw], zT01[:, sl0],
                                         wo01[:, n0:n0 + nw],
                                         start=True, stop=False)
                        nc.tensor.matmul(ps1[:, n0:n0 + nw], zT01[:, sl1],
                                         wo01[:, n0:n0 + nw],
                                         start=True, stop=False)
                    for n0, nw in ((0, 512), (512, 256)):
                        # zT2 K=64: even tile on rows 0-63, odd on 64-127 ->
                        # row-group paired
                        nc.tensor.matmul(ps0[:, n0:n0 + nw], zT2x[0:64, sl0],
                                         wo2[0:64, n0:n0 + nw],
                                         start=False, stop=True)
                        nc.tensor.matmul(ps1[:, n0:n0 + nw],
                                         zT2x[64:128, sl1],
                                         wo2[64:128, n0:n0 + nw],
                                         start=False, stop=True)
                    for t, ps in ((t0, ps0), (t1, ps1)):
                        ob = out_pool.tile([128, D], F32, tag="ob")
                        if qi == NQB - 1:
                            # last block: ACT is done with exps, split halves
                            nc.vector.tensor_copy(ob[:, 0:384], ps[:, 0:384])
                            nc.scalar.copy(ob[:, 384:D], ps[:, 384:D])
                        else:
                            nc.vector.tensor_copy(ob[:], ps[:])
                        nc.sync.dma_start(out_d[t * 128:(t + 1) * 128, :], ob[:])
                return emit

            for qi in range(NQB):
                q0 = qi * QB
                J = 4 * qi + 4
                qsl = slice(q0, q0 + QB)

                def exp_mask(rr, st, name):
                    """exp+mask a [128, 1024] score tile whose halves are
                    k-tiles with diagonal offsets rr=(r0, r1); r<0 = fully
                    below diagonal (no masking). Memsets go to the otherwise
                    idle GpSimd engine to keep DVE load down."""
                    pt = pt_pool.tile([128, 1024], BF16, tag="pt", name=name)
                    s0 = rr[0] * 128 if rr[0] >= 0 else 0
                    nc.scalar.activation(pt[:, s0:1024], st[:, s0:1024], AF.Exp)
                    for jj, r in enumerate(rr):
                        off = jj * 512
                        if r >= 0:
                            if r > 0:
                                nc.gpsimd.memset(pt[:, off:off + r * 128], 0.0)
                            dsl = slice(off + r * 128, off + (r + 1) * 128)
                            nc.vector.tensor_mul(pt[:, dsl], pt[:, dsl], tri[:])
                    return pt

                zts = [zT_pool.tile([128, 512], F32, tag="zT", name=f"zt{i}")
                       for i in range(3)]

                def pv_one(hv, j, pt, half):
                    nc.tensor.matmul(
                        zts[hv][:],
                        v_sb[:, j * 384 + hv * 128:j * 384 + (hv + 1) * 128],
                        pt[:, half * 512:(half + 1) * 512],
                        start=(j == 0), stop=(j == J - 1))

                # Two PV queues with different skews. h0's PSUM slot is
                # always free at q-block entry, so its PVs follow their exp
                # closely. h1/h2's slots WAR on the PREVIOUS block's
                # normalize (DRAM-bounce, ~10us): defer those PVs ~SKEW
                # tiles so they never block the PE queue (and the exp
                # stream) while the bounce is in flight.
                SKEW = 10
                q_fast = []   # (hv, j, pt, half) ready next round
                q_slow = []   # (tile_ridx, hv, j, pt, half)

                rounds = [("p", j) for j in range(J)] + \
                         [("2", jp) for jp in range(J // 2)]
                n_r = len(rounds)
                # previous block's out-proj pairs go ~12/15 tiles in (the
                # normalize bounce has landed by then), clamped to this
                # block's length
                op_sched = []
                if deferred_op is not None:
                    op_sched = [[min(12, n_r - 2), 0, deferred_op],
                                [min(15, n_r - 1), 1, deferred_op]]
                    deferred_op = None

                for ridx, (kind, idx) in enumerate(rounds):
                    for ent in op_sched:
                        if ent is not None and ent[0] == ridx:
                            ent[2](ent[1])
                            op_sched[op_sched.index(ent)] = None
                    op_sched = [e for e in op_sched if e is not None]
                    st = sT_pool.tile([128, 1024], F32, tag="sT",
                                      name=f"st_{kind}{idx}")
                    if kind == "p":
                        j = idx
                        ksl = slice(j * 128, (j + 1) * 128)
                        # both halves of one k-tile: rows 0-63 (h0) and
                        # 64-127 (h1) -> row-group paired in the PE
                        nc.tensor.matmul(st[:, 0:512], KT01[0:64, ksl],
                                         QT01[0:64, qsl], start=True, stop=True)
                        nc.tensor.matmul(st[:, 512:1024], KT01[64:128, ksl],
                                         QT01[64:128, qsl], start=True, stop=True)
                        rr = (idx - 4 * qi, idx - 4 * qi)
                    else:
                        j0, j1 = 2 * idx, 2 * idx + 1
                        nc.tensor.matmul(st[:, 0:512],
                                         QK2x[0:64, j0 * 128:(j0 + 1) * 128],
                                         QKT2[0:64, qsl], start=True, stop=True)
                        nc.tensor.matmul(st[:, 512:1024],
                                         QKT2[64:128, j1 * 128:(j1 + 1) * 128],
                                         QK2x[64:128, qsl], start=True, stop=True)
                        rr = (j0 - 4 * qi, j1 - 4 * qi)

                    # flush ready PVs: fast queue (skew 1), slow queue (SKEW)
                    for task in q_fast:
                        pv_one(*task)
                    q_fast = []
                    while q_slow and q_slow[0][0] <= ridx - SKEW:
                        pv_one(*q_slow.pop(0)[1:])

                    pt = exp_mask(rr, st, f"pt{kind}{idx}")
                    if kind == "p":
                        q_fast.append((0, idx, pt, 0))
                        q_slow.append((ridx, 1, idx, pt, 1))
                    else:
                        q_slow.append((ridx, 2, 2 * idx, pt, 0))
                        q_slow.append((ridx, 2, 2 * idx + 1, pt, 1))

                for task in q_fast:
                    pv_one(*task)
                for task in q_slow:
                    pv_one(*task[1:])
                for ent in op_sched:
                    ent[2](ent[1])

                # normalize: zT_h = zt_h[0:64] * broadcast(1 / zt_h[64]).
                # The reciprocal of the 3x512 sums runs on a [128, 12]
                # reshape (via a DRAM bounce) — InstReciprocal costs ~6.5ns
                # per FREE element, so a [64, 512] layout would cost 3.3us.
                # The bounce latency (~8us) hides under the next q-block's
                # score/exp stream (sT rotation is independent of it).
                s3 = recip_pool.tile([1, 3 * 512], F32, tag="s3")
                for h in range(3):
                    nc.vector.tensor_copy(s3[:, h * 512:(h + 1) * 512],
                                          zts[h][64:65, :])
                dr1 = rdr_pool.tile([1, 3 * 512], F32, tag="dr1")
                nc.sync.dma_start(dr1[:], s3[:])
                rs = recip_pool.tile([128, 12], F32, tag="rs")
                nc.sync.dma_start(
                    rs[:], dr1[:].rearrange("o (p f) -> (o p) f", p=128))
                rr_t = recip_pool.tile([128, 12], F32, tag="rr")
                nc.vector.reciprocal(rr_t[:], rs[:])
                dr2 = rdr_pool.tile([1, 3 * 512], F32, tag="dr2")
                nc.sync.dma_start(
                    dr2[:].rearrange("o (p f) -> (o p) f", p=128), rr_t[:])
                rb = rb_pool.tile([64, 3 * 512], F32, tag="rb")
                for h in range(3):
                    nc.sync.dma_start(
                        rb[:, h * 512:(h + 1) * 512],
                        dr2[0:1, h * 512:(h + 1) * 512].broadcast_to([64, 512]))

                nc.vector.tensor_mul(zT01[0:64, qsl], zts[0][0:64, :],
                                     rb[:, 0:512])
                z1 = zs_pool.tile([64, 512], BF16, tag="z1")
                nc.vector.tensor_mul(z1[:], zts[1][0:64, :], rb[:, 512:1024])
                # head 1 lives on partitions 64-127 of zT01: DMA partition-shift
                nc.sync.dma_start(zT01[64:128, qsl], z1[:])
                z2 = zs_pool.tile([64, 512], BF16, tag="z2")
                nc.vector.tensor_mul(z2[:], zts[2][0:64, :], rb[:, 1024:1536])
                # z_h2 on both partition halves for out-proj row-group pairing
                nc.sync.dma_start(zT2x[0:64, qsl], z2[:])
                nc.sync.dma_start(zT2x[64:128, qsl], z2[:])

                deferred_op = make_op_emitter(qi)

            # last q-block's output projection (ACT is free: split copies)
            deferred_op(0)
            deferred_op(1)

    nc.compile()
    return nc


def _get_nc():
    global _BUILT
    if _BUILT is None:
        _BUILT = build_nc()
    return _BUILT


def make_in_maps(inputs):
    x = np.asarray(inputs["normalized_resid_pre"], dtype=np.float32)
    W_Q = np.asarray(inputs["W_Q"], dtype=np.float32)
    W_K = np.asarray(inputs["W_K"], dtype=np.float32)
    W_V = np.asarray(inputs["W_V"], dtype=np.float32)
    W_O = np.asarray(inputs["W_O"], dtype=np.float32)
    b_Q = np.asarray(inputs["b_Q"], dtype=np.float32)
    b_K = np.asarray(inputs["b_K"], dtype=np.float32)
    b_V = np.asarray(inputs["b_V"], dtype=np.float32)
    sc = 1.0 / np.sqrt(np.float32(DH))

    in_maps = []
    for c in range(N_CORES):
        b = c // 4
        h = (c % 4) * 3
        hs = [h, h + 1, h + 2]
        m = {
            "xT": np.ascontiguousarray(x[b].T).astype(NPBF16),
            "wq01": np.concatenate([W_Q[hs[0]] * sc, W_Q[hs[1]] * sc],
                                   axis=1).astype(NPBF16),
            "wk01": np.concatenate([W_K[hs[0]], W_K[hs[1]]], axis=1).astype(NPBF16),
            "wqk2": np.concatenate([W_Q[hs[2]] * sc, W_K[hs[2]]],
                                   axis=1).astype(NPBF16),
            "wv": np.concatenate([W_V[hh] for hh in hs],
                                 axis=1).astype(NPBF16),
            "wo01": np.concatenate([W_O[hs[0]], W_O[hs[1]]], axis=0).astype(NPBF16),
            "wo2": np.concatenate([W_O[hs[2]], W_O[hs[2]]], axis=0).astype(NPBF16),
            "bq01": (np.concatenate([b_Q[hs[0]], b_Q[hs[1]]]) * sc)[:, None]
                    .astype(np.float32),
            "bk01": np.concatenate([b_K[hs[0]], b_K[hs[1]]])[:, None]
                    .astype(np.float32),
            "bqk2": np.concatenate([b_Q[hs[2]] * sc, b_K[hs[2]]])[:, None]
                    .astype(np.float32),
            "bv": np.ascontiguousarray(np.broadcast_to(
                np.concatenate([b_V[hh] for hh in hs]),
                (128, 192))).astype(np.float32),
        }
        in_maps.append(m)
    return in_maps


def kernel(**inputs):
    global LAST_EXEC_NS, LAST_RESULT
    nc = _get_nc()
    in_maps = make_in_maps(inputs)
    b_O = np.asarray(inputs["b_O"], dtype=np.float32)

    res = run_bass_kernel_spmd(nc, in_maps, core_ids=list(range(N_CORES)),
                               trace=TRACE_ENABLED)
    LAST_EXEC_NS = res.exec_time_ns
    LAST_RESULT = res
    parts = [r["out_p"] for r in res.results]
    out0 = parts[0] + parts[1] + parts[2] + parts[3]
    out1 = parts[4] + parts[5] + parts[6] + parts[7]
    out = np.stack([out0, out1]) + b_O
    return out.astype(np.float32)
